# revision 1
# baseline (speedup 1.0000x reference)
"""Multi-head causal self-attention with RoPE on 8 TRN2 NeuronCores.

Sharding: tensor-parallel over heads. Each core owns 2 of the 16 heads:
it gets the matching rows of W_qkv and columns of W_o (host-sliced,
host-transposed, bf16-cast), computes a full [B*L, D] partial output, and
the host sums the 8 partials (the o_proj all-reduce).

Per-core pipeline (all matmuls bf16 with fp32 PSUM accumulation):
  x --DMA-xbar-transpose--> xT chunks [d, t]
  xT (stationary) @ WqkvT (moving) -> q,k,v in natural [t, e] layout
  RoPE on q,k in natural layout (pair swap = free-dim strided copies)
  PE-transpose q,k -> qT,kT [dh, t]
  S^T[k,q] = kT^T @ qT   (softmax needs no transposes in this layout;
                          no max-subtraction: |scores*scale| <~ 4)
  p^T = exp(scale*S^T) * diag_mask;  l = ones^T @ p^T (PE column sums)
  y^T[dh,q] = v_nat^T @ p^T,  scaled by 1/l (gpsimd partition-broadcast)
  out[t,e] = y^T^T @ WoT  -> fp32 partial
"""

import sys

if "/opt/trn_rl_repo" not in sys.path:
    sys.path.insert(0, "/opt/trn_rl_repo")

import math

import ml_dtypes
import numpy as np

import concourse.bass as bass
import concourse.mybir as mybir
import concourse.tile as tile
from concourse.bass_utils import run_bass_kernel_spmd
from concourse.vector_clock import ScopedClock

BF16 = ml_dtypes.bfloat16
FP32 = mybir.dt.float32
BF = mybir.dt.bfloat16

ROPE_THETA = 10000.0


def _split_multi_waits(nc):
    """This container's walrus build rejects >1 sync wait per instruction.
    Hoist all but one wait of each instruction onto same-engine NOPs placed
    immediately before it (same-engine program order makes this equivalent)."""
    for f in nc.m.functions:
        for bb in f.blocks:
            insts = bb.instructions
            if not any(
                i.sync_info is not None and len(i.sync_info.on_wait) > 1
                for i in insts
            ):
                continue
            new = []
            for inst in insts:
                si = inst.sync_info
                if si is not None and len(si.on_wait) > 1:
                    waits = list(si.on_wait)
                    si.on_wait.clear()
                    si.on_wait.append(waits[0])
                    for idx, w in enumerate(waits[1:]):
                        new.append(
                            mybir.InstNoOp(
                                name=f"{inst.name}-sw{idx}",
                                engine=inst.engine,
                                bass_nofuse=True,
                                sync_info=mybir.SyncInfo(on_wait=[w], on_update=[]),
                            )
                        )
                new.append(inst)
            bb.instructions = new


class TileContextSplitDrain(tile.TileContext):
    """TileContext adapted to this walrus build's 1-sync-wait-per-instruction
    limit: split the tail drain's waits and run _split_multi_waits over the
    whole scheduled program."""

    def _drain_and_barrier(self, tick_clock, wait_clock):
        _split_multi_waits(self.nc)
        drain_inst = self.nc.sync.drain()
        wait_clock.add_sem_waits(
            drain_inst.ins, ScopedClock({None: tick_clock.global_clock})
        )
        waits = list(drain_inst.ins.sync_info.on_wait)
        if len(waits) > 1:
            drain_inst.ins.sync_info.on_wait.clear()
            drain_inst.ins.sync_info.on_wait.append(waits[0])
            for w in waits[1:]:
                nop = self.nc.sync.nop(nofuse=True)
                if nop.ins.sync_info is None:
                    nop.ins.sync_info = mybir.SyncInfo(on_update=[], on_wait=[])
                nop.ins.sync_info.on_wait.append(w)

        self.nc.all_engine_barrier()
        assert self.sems is not None
        popped = self.nc._tile_sem_poison_stack.pop()
        assert popped is self._sem_poison
        self.nc.clear_and_free_semaphores(list(self.sems.allocated().values()))
        self.nc.all_engine_barrier()


def _bcast_mid(ap, rep):
    """[128, N] AP -> [128, rep, N] AP with a stride-0 middle dim."""
    return bass.AP(ap.tensor, ap.offset, [ap.ap[0], [0, rep], ap.ap[1]])


def build_core_kernel(B, L, D, HL, dh=128, TCH=512, QC=512):
    """One core's kernel: HL local heads over the full [B*L, D] input."""
    T = B * L
    DT = D // 128          # d-tiles
    LT = L // 128          # l-tiles per batch
    EQK = 2 * HL * dh      # q+k columns in wqkvT
    EV = HL * dh           # v columns
    NCH = T // TCH         # t-chunks
    TPC = TCH // 128       # t-tiles per chunk
    NQC = L // QC          # q-chunks per (b, h)
    NDIAG = QC // 128
    scale = 1.0 / math.sqrt(dh)

    nc = bass.Bass("TRN2", target_bir_lowering=False)
    xbT = nc.dram_tensor("xbT", [D, T], BF, kind="ExternalInput")
    wqkvT = nc.dram_tensor("wqkvT", [D, EQK + EV], BF, kind="ExternalInput")
    woT = nc.dram_tensor("woT", [EV, D], BF, kind="ExternalInput")
    cosn = nc.dram_tensor("cosn", [128, LT, dh], FP32, kind="ExternalInput")
    sinn = nc.dram_tensor("sinn", [128, LT, dh], FP32, kind="ExternalInput")
    masks = nc.dram_tensor("masks", [NDIAG, 128, QC], BF, kind="ExternalInput")
    ident = nc.dram_tensor("ident", [128, 128], BF, kind="ExternalInput")
    out = nc.dram_tensor("out", [T, D], FP32, kind="ExternalOutput")

    with TileContextSplitDrain(nc) as tc:
        with (
            tc.tile_pool(name="static", bufs=1) as st,
            tc.tile_pool(name="xt", bufs=2) as xt_pool,
            tc.tile_pool(name="ropef", bufs=3) as ropef,
            tc.tile_pool(name="qknat", bufs=3) as qknat_pool,
            tc.tile_pool(name="pt", bufs=4) as pt_pool,
            tc.tile_pool(name="small", bufs=4) as small,
            tc.tile_pool(name="ostage", bufs=4) as ostage,
            tc.tile_pool(name="psum_proj", bufs=2, space="PSUM") as ps_proj,
            tc.tile_pool(name="psum_ts", bufs=2, space="PSUM") as ps_ts,
            tc.tile_pool(name="psum_y", bufs=2, space="PSUM") as ps_y,
            tc.tile_pool(name="psum_l", bufs=2, space="PSUM") as ps_l,
        ):
            # --- static loads (scalar HWDGE ring; sync ring is reserved for
            # the x transpose stream to avoid xbar-mode transitions) ---
            wqv = wqkvT.rearrange("(dt p) e -> p dt e", p=128)
            wq_sb = st.tile([128, DT, EQK + EV], BF)
            for dt in range(DT):
                nc.scalar.dma_start(wq_sb[:, dt, :], wqv[:, dt, :])
            wo_sb = st.tile([128, HL, D], BF)
            nc.scalar.dma_start(wo_sb[:], woT.rearrange("(h p) e -> p h e", p=128))
            cos_sb = st.tile([128, LT, dh], FP32)
            nc.scalar.dma_start(cos_sb[:], cosn[:])
            sin_sb = st.tile([128, LT, dh], FP32)
            nc.scalar.dma_start(sin_sb[:], sinn[:])
            masks_sb = st.tile([128, 128], BF)
            nc.scalar.dma_start(masks_sb[:], masks[0, :, 0:128])
            ident_sb = st.tile([128, 128], BF)
            nc.scalar.dma_start(ident_sb[:], ident[:])
            ones_mat = st.tile([128, 128], BF)
            nc.vector.memset(ones_mat[:], 1.0)

            qT_sb = st.tile([128, HL, B, L], BF)
            kT_sb = st.tile([128, HL, B, L], BF)
            v_sb = st.tile([128, T // 128, EV], BF)
            yT_sb = st.tile([128, HL, B, L], BF)

            # --- phase A: qkv projection + rope + q/k transpose ---
            copy_flip = 0
            for tch in range(NCH):
                b = (tch * TCH) // L
                l0 = (tch * TCH) % L
                xT_c = xt_pool.tile([128, DT, TCH], BF)
                for dt in range(DT):
                    nc.sync.dma_start(
                        out=xT_c[:, dt, :],
                        in_=xbT[dt * 128:(dt + 1) * 128, tch * TCH:(tch + 1) * TCH],
                    )
                for tt in range(TPC):
                    lt = (l0 + tt * 128) // 128
                    tstat = [xT_c[:, dt, tt * 128:(tt + 1) * 128] for dt in range(DT)]
                    # q,k natural [t, e]
                    ps_qk = ps_proj.tile([128, EQK], FP32, tag="psum_proj")
                    for dt in range(DT):
                        nc.tensor.matmul(
                            ps_qk[:], tstat[dt], wq_sb[:, dt, 0:EQK],
                            start=(dt == 0), stop=(dt == DT - 1),
                        )
                    # rope: sw = pair-swapped psum (free-strided copies)
                    sw = ropef.tile([128, EQK], FP32, tag="sw")
                    ps3 = ps_qk.rearrange("p (c two) -> p c two", two=2)
                    sw3 = sw.rearrange("p (c two) -> p c two", two=2)
                    nc.scalar.activation(
                        sw3[:, :, 0], ps3[:, :, 1], mybir.ActivationFunctionType.Copy
                    )
                    nc.vector.tensor_copy(sw3[:, :, 1], ps3[:, :, 0])
                    cos_b = _bcast_mid(cos_sb[:, lt, :], 2 * HL)
                    sin_b = _bcast_mid(sin_sb[:, lt, :], 2 * HL)
                    tA = ropef.tile([128, EQK], FP32, tag="tA")
                    tB = ropef.tile([128, EQK], FP32, tag="tB")
                    nc.vector.tensor_mul(
                        tA.rearrange("p (h e) -> p h e", e=dh), ps_qk.rearrange("p (h e) -> p h e", e=dh), cos_b
                    )
                    nc.vector.tensor_mul(
                        tB.rearrange("p (h e) -> p h e", e=dh), sw.rearrange("p (h e) -> p h e", e=dh), sin_b
                    )
                    qk_nat = qknat_pool.tile([128, EQK], BF)
                    nc.vector.tensor_add(qk_nat[:], tA[:], tB[:])
                    # transpose q,k slices -> qT/kT
                    ps_t = ps_ts.tile([128, EQK], BF, tag="ps_t")
                    for j in range(2 * HL):
                        nc.tensor.transpose(
                            ps_t[:, j * 128:(j + 1) * 128],
                            qk_nat[:, j * 128:(j + 1) * 128],
                            ident_sb[:],
                        )
                    lsl = slice(l0 + tt * 128, l0 + (tt + 1) * 128)
                    q_dst = qT_sb[:, :, b, lsl]
                    k_dst = kT_sb[:, :, b, lsl]
                    q_src = ps_t[:, 0:HL * 128].rearrange("p (h t) -> p h t", h=HL)
                    k_src = ps_t[:, HL * 128:2 * HL * 128].rearrange(
                        "p (h t) -> p h t", h=HL
                    )
                    if copy_flip % 2 == 0:
                        nc.scalar.activation(
                            q_dst, q_src, mybir.ActivationFunctionType.Copy
                        )
                        nc.vector.tensor_copy(k_dst, k_src)
                    else:
                        nc.vector.tensor_copy(q_dst, q_src)
                        nc.scalar.activation(
                            k_dst, k_src, mybir.ActivationFunctionType.Copy
                        )
                    copy_flip += 1
                    # v natural [t, e]
                    ps_v = ps_proj.tile([128, EV], FP32, tag="psum_proj")
                    for dt in range(DT):
                        nc.tensor.matmul(
                            ps_v[:], tstat[dt], wq_sb[:, dt, EQK:EQK + EV],
                            start=(dt == 0), stop=(dt == DT - 1),
                        )
                    nc.vector.tensor_copy(v_sb[:, tch * TPC + tt, :], ps_v[:])

            # --- phase B (attention) and C (o_proj) interleaved per batch so
            # each batch's o_proj stores overlap the next batch's attention ---
            def phase_b(b):
                for h in range(HL):
                    for qc in range(NQC):
                        nk = (qc + 1) * QC // 128
                        ps_yt = ps_y.tile([128, QC], FP32)
                        # column sums of p^T land replicated across all 128
                        # partitions (ones-matrix stationary) so the
                        # reciprocal runs wide, not on one partition
                        ps_lt = ps_l.tile([128, QC], FP32)
                        for kb in range(nk):
                            # diagonal blocks only produce q >= kb*128: trim
                            # the moving range; only the first 128 columns of
                            # the trimmed region are triangular
                            q_lo = max(0, kb * 128 - qc * QC)
                            cs = slice(q_lo, QC)
                            qmov = qT_sb[:, h, b, qc * QC + q_lo:(qc + 1) * QC]
                            ps_s = ps_ts.tile([128, QC], FP32, tag="ps_t")
                            nc.tensor.matmul(
                                ps_s[:, cs],
                                kT_sb[:, h, b, kb * 128:(kb + 1) * 128],
                                qmov,
                                start=True, stop=True,
                            )
                            pT = pt_pool.tile([128, QC], BF)
                            nc.scalar.activation(
                                pT[:, cs], ps_s[:, cs],
                                mybir.ActivationFunctionType.Exp, scale=scale,
                            )
                            if kb >= NDIAG * qc:
                                nc.vector.tensor_mul(
                                    pT[:, q_lo:q_lo + 128],
                                    pT[:, q_lo:q_lo + 128],
                                    masks_sb[:],
                                )
                            nc.tensor.matmul(
                                ps_yt[:, cs],
                                v_sb[:, b * LT + kb, h * dh:(h + 1) * dh],
                                pT[:, cs],
                                start=(kb == 0), stop=(kb == nk - 1),
                            )
                            nc.tensor.matmul(
                                ps_lt[:, cs], ones_mat[:], pT[:, cs],
                                start=(kb == 0), stop=(kb == nk - 1),
                            )
                        # 1/l as exp(-ln(l)) on the scalar engine: DVE's
                        # reciprocal is ~3.4us per call and custom-DVE ops
                        # don't compile here; l in [1, 2.6e3] is safely inside
                        # the table range
                        lnl = small.tile([128, QC], FP32, tag="lnl")
                        nc.scalar.activation(
                            lnl[:], ps_lt[:], mybir.ActivationFunctionType.Ln
                        )
                        invb = small.tile([128, QC], FP32, tag="invb")
                        nc.scalar.activation(
                            invb[:], lnl[:],
                            mybir.ActivationFunctionType.Exp, scale=-1.0,
                        )
                        nc.vector.tensor_mul(
                            yT_sb[:, h, b, qc * QC:(qc + 1) * QC], ps_yt[:], invb[:]
                        )

            def phase_c(b):
                for lt in range(LT):
                    ttg = b * LT + lt
                    for ec in range(D // 512):
                        ps_o = ps_proj.tile([128, 512], FP32, tag="psum_proj")
                        for h in range(HL):
                            nc.tensor.matmul(
                                ps_o[:],
                                yT_sb[:, h, b, lt * 128:(lt + 1) * 128],
                                wo_sb[:, h, ec * 512:(ec + 1) * 512],
                                start=(h == 0), stop=(h == HL - 1),
                            )
                        ot = ostage.tile([128, 512], FP32)
                        nc.vector.tensor_copy(ot[:], ps_o[:])
                        nc.sync.dma_start(
                            out[ttg * 128:(ttg + 1) * 128, ec * 512:(ec + 1) * 512],
                            ot[:],
                        )

            for b in range(B):
                phase_b(b)
            for b in range(B):
                phase_c(b)
    return nc


def _rope_tables(L, dh, LT):
    inv_freq = 1.0 / (ROPE_THETA ** (np.arange(0, dh, 2, dtype=np.float32) / dh))
    ang = np.arange(L, dtype=np.float32)[:, None] * inv_freq[None, :]  # [L, dh/2]
    cos = np.repeat(np.cos(ang), 2, axis=-1)                          # [L, dh]
    sin = np.repeat(np.sin(ang), 2, axis=-1)
    sgn = np.where(np.arange(dh) % 2 == 0, -1.0, 1.0).astype(np.float32)
    sinn = sin * sgn[None, :]
    # [L, dh] -> [128, LT, dh] with partition = l % 128
    cosn = np.ascontiguousarray(
        cos.reshape(LT, 128, dh).transpose(1, 0, 2)
    ).astype(np.float32)
    sinn = np.ascontiguousarray(
        sinn.reshape(LT, 128, dh).transpose(1, 0, 2)
    ).astype(np.float32)
    return cosn, sinn


def _diag_masks(QC):
    nd = QC // 128
    m = np.zeros((nd, 128, QC), dtype=BF16)
    p = np.arange(128)[:, None]
    f = np.arange(QC)[None, :]
    for j in range(nd):
        m[j] = (p + 128 * j <= f).astype(BF16)
    return m


def make_in_maps(x, W_qkv, W_o, n_cores=8, H=16):
    B, L, D = x.shape
    T = B * L
    dh = D // H
    HL = H // n_cores
    LT = L // 128
    xbfT = np.ascontiguousarray(x.reshape(T, D).T).astype(BF16)
    cosn, sinn = _rope_tables(L, dh, LT)
    masks = _diag_masks(512)
    identity = np.eye(128, dtype=BF16)
    in_maps = []
    for c in range(n_cores):
        r0 = c * HL * dh
        r1 = (c + 1) * HL * dh
        wl = np.concatenate(
            [W_qkv[r0:r1], W_qkv[D + r0:D + r1], W_qkv[2 * D + r0:2 * D + r1]], axis=0
        )
        wqkvT = np.ascontiguousarray(wl.T).astype(BF16)
        woT = np.ascontiguousarray(W_o[:, r0:r1].T).astype(BF16)
        in_maps.append(
            {
                "xbT": xbfT,
                "wqkvT": wqkvT,
                "woT": woT,
                "cosn": cosn,
                "sinn": sinn,
                "masks": masks,
                "ident": identity,
            }
        )
    return in_maps


_NC_CACHE = {}


def _get_nc(B, L, D, HL):
    key = (B, L, D, HL)
    if key not in _NC_CACHE:
        _NC_CACHE[key] = build_core_kernel(B, L, D, HL)
    return _NC_CACHE[key]


def kernel(x, W_qkv, W_o, trace=False):
    x = np.asarray(x)
    W_qkv = np.asarray(W_qkv)
    W_o = np.asarray(W_o)
    B, L, D = x.shape
    n_cores, H = 8, 16
    HL = H // n_cores
    nc = _get_nc(B, L, D, HL)
    in_maps = make_in_maps(x, W_qkv, W_o, n_cores=n_cores, H=H)
    res = run_bass_kernel_spmd(
        nc, in_maps, core_ids=list(range(n_cores)), trace=trace
    )
    acc = np.zeros((B * L, D), dtype=np.float64)
    for r in res.results:
        acc += r["out"].astype(np.float64)
    out = acc.astype(np.float32).reshape(B, L, D)
    if trace:
        return out, res
    return out



# revision 2
# speedup vs baseline: 1.2026x; 1.2026x over previous
"""Multi-head causal self-attention with RoPE on 8 TRN2 NeuronCores.

Sharding: tensor-parallel over heads. Each core owns 2 of the 16 heads:
it gets the matching rows of W_qkv and columns of W_o (host-sliced,
host-transposed, fp16-cast), computes a full [B*L, D] partial output, and
the host sums the 8 partials (the o_proj all-reduce).

Per-core pipeline (all matmuls fp16 with fp32 PSUM accumulation):
  x --host-transposed--> xT chunks [d, t] (one batched DMA per chunk)
  xT (stationary) @ WqkvT (moving) -> q,k,v in natural [t, e] layout
  RoPE on q,k in natural layout over PAIRED t-tiles ([128,1024] ops)
  PE-transpose q,k -> qT,kT [dh, t]
  S^T[k,q] = kT^T @ qT into paired 2-bank PSUM tiles; causal mask applied
    on the PE itself (amask @ I accumulate, -1e4 bias) so exp needs no
    DVE mask hop; one wide exp per pair.
  p^T scores emitted one pair AHEAD of the y/l matmuls so the PE never
    waits on the scalar engine's exp.
  l = ones^T @ p^T (PE column sums, replicated across partitions)
  y^T[dh,q] = v_nat^T @ p^T, scaled by 1/l = exp(-ln(l))
  out[t,e] = y^T^T @ WoT -> fp16 partial, paired PSUM + wide copies,
    one 512KB DMA per t-tile; emission order B(0) C(0) B(1) C(1) so
    out-DMA of batch 0 overlaps attention of batch 1.
"""

import sys

if "/opt/trn_rl_repo" not in sys.path:
    sys.path.insert(0, "/opt/trn_rl_repo")

import math

import numpy as np

import concourse.bass as bass
import concourse.mybir as mybir
import concourse.tile as tile
from concourse.bass_utils import run_bass_kernel_spmd
from concourse.vector_clock import ScopedClock

F16 = np.float16
FP32 = mybir.dt.float32
HF = mybir.dt.float16

ROPE_THETA = 10000.0
MASK_NEG = -10000.0


def _split_multi_waits(nc):
    """This container's walrus build rejects >1 sync wait per instruction.
    Hoist all but one wait of each instruction onto same-engine NOPs placed
    immediately before it (same-engine program order makes this equivalent)."""
    for f in nc.m.functions:
        for bb in f.blocks:
            insts = bb.instructions
            if not any(
                i.sync_info is not None and len(i.sync_info.on_wait) > 1
                for i in insts
            ):
                continue
            new = []
            for inst in insts:
                si = inst.sync_info
                if si is not None and len(si.on_wait) > 1:
                    waits = list(si.on_wait)
                    si.on_wait.clear()
                    si.on_wait.append(waits[0])
                    for idx, w in enumerate(waits[1:]):
                        new.append(
                            mybir.InstNoOp(
                                name=f"{inst.name}-sw{idx}",
                                engine=inst.engine,
                                bass_nofuse=True,
                                sync_info=mybir.SyncInfo(on_wait=[w], on_update=[]),
                            )
                        )
                new.append(inst)
            bb.instructions = new


class TileContextSplitDrain(tile.TileContext):
    """TileContext adapted to this walrus build's 1-sync-wait-per-instruction
    limit: split the tail drain's waits and run _split_multi_waits over the
    whole scheduled program."""

    def _drain_and_barrier(self, tick_clock, wait_clock):
        _split_multi_waits(self.nc)
        drain_inst = self.nc.sync.drain()
        wait_clock.add_sem_waits(
            drain_inst.ins, ScopedClock({None: tick_clock.global_clock})
        )
        waits = list(drain_inst.ins.sync_info.on_wait)
        if len(waits) > 1:
            drain_inst.ins.sync_info.on_wait.clear()
            drain_inst.ins.sync_info.on_wait.append(waits[0])
            for w in waits[1:]:
                nop = self.nc.sync.nop(nofuse=True)
                if nop.ins.sync_info is None:
                    nop.ins.sync_info = mybir.SyncInfo(on_update=[], on_wait=[])
                nop.ins.sync_info.on_wait.append(w)

        self.nc.all_engine_barrier()
        assert self.sems is not None
        popped = self.nc._tile_sem_poison_stack.pop()
        assert popped is self._sem_poison
        self.nc.clear_and_free_semaphores(list(self.sems.allocated().values()))
        self.nc.all_engine_barrier()


def _ap4(ap, dims):
    """Raw AP with explicit [stride, size] dims after the partition dim."""
    return bass.AP(ap.tensor, ap.offset, [ap.ap[0]] + dims)


def build_core_kernel(B, L, D, HL, dh=128, TCH=512, QC=512):
    """One core's kernel: HL local heads over the full [B*L, D] input."""
    T = B * L
    DT = D // 128          # d-tiles
    LT = L // 128          # l-tiles per batch
    EQK = 2 * HL * dh      # q+k columns in wqkvT (512)
    EV = HL * dh           # v columns (256)
    NCH = T // TCH         # t-chunks
    TPC = TCH // 128       # t-tiles per chunk (4)
    NQC = L // QC          # q-chunks per (b, h)
    NDIAG = QC // 128
    scale = 1.0 / math.sqrt(dh)

    nc = bass.Bass("TRN2", target_bir_lowering=False)
    xbT = nc.dram_tensor("xbT", [D, T], HF, kind="ExternalInput")
    wqkvT = nc.dram_tensor("wqkvT", [D, EQK + EV], HF, kind="ExternalInput")
    woT = nc.dram_tensor("woT", [EV, D], HF, kind="ExternalInput")
    cosn = nc.dram_tensor("cosn", [128, LT, dh], HF, kind="ExternalInput")
    sinn = nc.dram_tensor("sinn", [128, LT, dh], HF, kind="ExternalInput")
    amask = nc.dram_tensor("amask", [128, 128], HF, kind="ExternalInput")
    ident = nc.dram_tensor("ident", [128, 128], HF, kind="ExternalInput")
    out = nc.dram_tensor("out", [T, D], HF, kind="ExternalOutput")

    with TileContextSplitDrain(nc) as tc:
        with (
            tc.tile_pool(name="static", bufs=1) as st,
            tc.tile_pool(name="xt", bufs=2) as xt_pool,
            tc.tile_pool(name="ropef", bufs=2) as ropef,
            tc.tile_pool(name="qknat", bufs=3) as qknat_pool,
            tc.tile_pool(name="pt", bufs=4) as pt_pool,
            tc.tile_pool(name="small", bufs=2) as small,
            tc.tile_pool(name="ostage", bufs=2) as ostage,
            # PSUM: 4 + 2 + 2 banks = all 8
            tc.tile_pool(name="ps_big", bufs=2, space="PSUM") as ps_big,
            tc.tile_pool(name="ps_y", bufs=2, space="PSUM") as ps_y,
            tc.tile_pool(name="ps_l", bufs=2, space="PSUM") as ps_l,
        ):
            # --- static loads; weights dt-sliced so the first matmul can
            # start as soon as slice 0 lands (x on sync ring, rest scalar) ---
            wqv = wqkvT.rearrange("(dt p) e -> p dt e", p=128)
            wq_sb = st.tile([128, DT, EQK + EV], HF)
            for dt in range(DT):
                nc.scalar.dma_start(wq_sb[:, dt, :], wqv[:, dt, :])
            wo_sb = st.tile([128, HL, D], HF)
            nc.scalar.dma_start(wo_sb[:], woT.rearrange("(h p) e -> p h e", p=128))
            cos_sb = st.tile([128, LT, dh], HF)
            nc.scalar.dma_start(cos_sb[:], cosn[:])
            sin_sb = st.tile([128, LT, dh], HF)
            nc.scalar.dma_start(sin_sb[:], sinn[:])
            amask_sb = st.tile([128, 128], HF)
            nc.scalar.dma_start(amask_sb[:], amask[:])
            ident_sb = st.tile([128, 128], HF)
            nc.scalar.dma_start(ident_sb[:], ident[:])
            ones_mat = st.tile([128, 128], HF)
            nc.vector.memset(ones_mat[:], 1.0)

            qT_sb = st.tile([128, HL, B, L], HF)
            kT_sb = st.tile([128, HL, B, L], HF)
            v_sb = st.tile([128, T // 128, EV], HF)
            yT_sb = st.tile([128, HL, B, L], HF)

            xbT_r = xbT.rearrange("(dt p) t -> p dt t", p=128)

            # --- phase A: qkv projection + rope + q/k transpose.
            # Work in PAIRS of t-tiles: qk matmuls fill a 2-bank psum tile,
            # rope runs as wide [128,1024] ops, v matmuls of the pair cover
            # the rope latency, and the transposes of the PREVIOUS pair are
            # emitted last so they never stall the PE. ---
            eng_flip = 0

            def eng(i):
                return nc.vector if i % 2 == 0 else nc.scalar

            def copy(engine, dst, src):
                if engine is nc.vector:
                    nc.vector.tensor_copy(dst, src)
                else:
                    nc.scalar.activation(dst, src, mybir.ActivationFunctionType.Copy)

            prev = None  # (qk_nat tile, b, l0 of even tile)

            def emit_transposes(pr):
                nonlocal eng_flip
                qk_nat, b, l0 = pr
                ps_tr = ps_l.tile([128, 1024], HF, tag="lt")
                for j in range(8):
                    nc.tensor.transpose(
                        ps_tr[:, j * 128:(j + 1) * 128],
                        qk_nat[:, j * 128:(j + 1) * 128],
                        ident_sb[:],
                    )
                # blocks: [q0 q1 k0 k1 | q0 q1 k0 k1] (even tile, odd tile)
                for half in range(2):
                    lsl = slice(l0 + half * 128, l0 + (half + 1) * 128)
                    q_src = ps_tr[:, half * 512:half * 512 + 256].rearrange(
                        "p (h t) -> p h t", h=HL
                    )
                    k_src = ps_tr[:, half * 512 + 256:half * 512 + 512].rearrange(
                        "p (h t) -> p h t", h=HL
                    )
                    copy(eng(eng_flip), qT_sb[:, :, b, lsl], q_src)
                    copy(eng(eng_flip + 1), kT_sb[:, :, b, lsl], k_src)
                    eng_flip += 1

            for tch in range(NCH):
                b = (tch * TCH) // L
                l0 = (tch * TCH) % L
                xT_c = xt_pool.tile([128, DT, TCH], HF)
                if tch == 0:
                    for dt in range(DT):
                        nc.sync.dma_start(xT_c[:, dt, :], xbT_r[:, dt, 0:TCH])
                else:
                    nc.sync.dma_start(
                        xT_c[:], xbT_r[:, :, tch * TCH:(tch + 1) * TCH]
                    )
                for pair in range(TPC // 2):
                    tt0 = 2 * pair
                    lt0 = (l0 + tt0 * 128) // 128
                    ps_qk = ps_big.tile([128, 1024], FP32, tag="mm1024")
                    for half in range(2):
                        tt = tt0 + half
                        for dt in range(DT):
                            nc.tensor.matmul(
                                ps_qk[:, half * 512:(half + 1) * 512],
                                xT_c[:, dt, tt * 128:(tt + 1) * 128],
                                wq_sb[:, dt, 0:EQK],
                                start=(dt == 0), stop=(dt == DT - 1),
                            )
                    # v natural [t, e] for both tiles of the pair
                    ps_vs = []
                    for half in range(2):
                        tt = tt0 + half
                        ps_v = ps_y.tile([128, 512], FP32, tag="yt")
                        for dt in range(DT):
                            nc.tensor.matmul(
                                ps_v[:, 0:EV],
                                xT_c[:, dt, tt * 128:(tt + 1) * 128],
                                wq_sb[:, dt, EQK:EQK + EV],
                                start=(dt == 0), stop=(dt == DT - 1),
                            )
                        ps_vs.append(ps_v)
                    # transposes of the previous pair (PE busy while rope of
                    # THIS pair runs on scalar/vector)
                    if prev is not None:
                        emit_transposes(prev)
                    # rope over the [128, 1024] pair: sw = pair-swapped psum
                    sw = ropef.tile([128, 1024], HF, tag="sw")
                    ps3 = ps_qk.rearrange("p (c two) -> p c two", two=2)
                    sw3 = sw.rearrange("p (c two) -> p c two", two=2)
                    nc.scalar.activation(
                        sw3[:, :, 0], ps3[:, :, 1], mybir.ActivationFunctionType.Copy
                    )
                    nc.scalar.activation(
                        sw3[:, :, 1], ps3[:, :, 0], mybir.ActivationFunctionType.Copy
                    )
                    # cos/sin for the pair: [128, tile(2), head(2HL), dh]
                    lt_stride = dh  # cos_sb free layout [LT, dh]
                    cos_b = _ap4(
                        cos_sb[:, lt0, :],
                        [[lt_stride, 2], [0, 2 * HL], [1, dh]],
                    )
                    sin_b = _ap4(
                        sin_sb[:, lt0, :],
                        [[lt_stride, 2], [0, 2 * HL], [1, dh]],
                    )
                    tA = ropef.tile([128, 1024], HF, tag="tA")
                    tB = ropef.tile([128, 1024], HF, tag="tB")
                    nc.vector.tensor_mul(
                        tA.rearrange("p (t h e) -> p t h e", t=2, e=dh),
                        ps_qk.rearrange("p (t h e) -> p t h e", t=2, e=dh),
                        cos_b,
                    )
                    nc.vector.tensor_mul(
                        tB.rearrange("p (t h e) -> p t h e", t=2, e=dh),
                        sw.rearrange("p (t h e) -> p t h e", t=2, e=dh),
                        sin_b,
                    )
                    qk_nat = qknat_pool.tile([128, 1024], HF)
                    nc.vector.tensor_add(qk_nat[:], tA[:], tB[:])
                    # v copies (alternate engines)
                    for half in range(2):
                        tt = tt0 + half
                        copy(
                            eng(eng_flip),
                            v_sb[:, tch * TPC + tt, :],
                            ps_vs[half][:, 0:EV],
                        )
                        eng_flip += 1
                    prev = (qk_nat, b, l0 + tt0 * 128)
            emit_transposes(prev)

            # --- phase B (attention): scores one PAIR ahead of y/l so the
            # PE never waits on exp; causal mask via PE accumulate. ---
            def phase_b(b):
                for h in range(HL):
                    for qc in range(NQC):
                        nk = (qc + 1) * NDIAG
                        P = nk // 2
                        ps_yt = ps_y.tile([128, QC], FP32, tag="yt")
                        ps_lt = ps_l.tile([128, QC], FP32, tag="lt")
                        pts = [None] * P
                        qlos = [None] * P

                        def emit_scores(p):
                            ps_s = ps_big.tile([128, 1024], FP32, tag="mm1024")
                            pt = pt_pool.tile([128, 1024], HF)
                            lo_all = None
                            for half in range(2):
                                kb = 2 * p + half
                                q_lo = max(0, kb * 128 - qc * QC)
                                if lo_all is None:
                                    lo_all = half * 512 + q_lo
                                diag = kb >= NDIAG * qc
                                nc.tensor.matmul(
                                    ps_s[:, half * 512 + q_lo:(half + 1) * 512],
                                    kT_sb[:, h, b, kb * 128:(kb + 1) * 128],
                                    qT_sb[:, h, b, qc * QC + q_lo:(qc + 1) * QC],
                                    start=True, stop=(not diag),
                                )
                                if diag:
                                    nc.tensor.matmul(
                                        ps_s[:, half * 512 + q_lo:half * 512 + q_lo + 128],
                                        amask_sb[:],
                                        ident_sb[:],
                                        start=False, stop=True,
                                    )
                            nc.scalar.activation(
                                pt[:, lo_all:1024], ps_s[:, lo_all:1024],
                                mybir.ActivationFunctionType.Exp, scale=scale,
                            )
                            pts[p] = pt
                            qlos[p] = lo_all

                        def emit_yl(p):
                            pt = pts[p]
                            for half in range(2):
                                kb = 2 * p + half
                                q_lo = max(0, kb * 128 - qc * QC)
                                cs = slice(half * 512 + q_lo, (half + 1) * 512)
                                nc.tensor.matmul(
                                    ps_yt[:, q_lo:QC],
                                    v_sb[:, b * LT + kb, h * dh:(h + 1) * dh],
                                    pt[:, cs],
                                    start=(kb == 0), stop=(kb == nk - 1),
                                )
                                nc.tensor.matmul(
                                    ps_lt[:, q_lo:QC], ones_mat[:], pt[:, cs],
                                    start=(kb == 0), stop=(kb == nk - 1),
                                )

                        for p in range(P):
                            emit_scores(p)
                            if p >= 1:
                                emit_yl(p - 1)
                        emit_yl(P - 1)
                        # 1/l as exp(-ln(l)); l replicated across partitions
                        lnl = small.tile([128, QC], FP32, tag="lnl")
                        nc.scalar.activation(
                            lnl[:], ps_lt[:], mybir.ActivationFunctionType.Ln
                        )
                        invb = small.tile([128, QC], FP32, tag="invb")
                        nc.scalar.activation(
                            invb[:], lnl[:],
                            mybir.ActivationFunctionType.Exp, scale=-1.0,
                        )
                        nc.vector.tensor_mul(
                            yT_sb[:, h, b, qc * QC:(qc + 1) * QC], ps_yt[:], invb[:]
                        )

            # --- phase C (o_proj): paired psum, wide fp16 copies, one DMA
            # per t-tile ---
            def phase_c(b):
                c_flip = 0
                for lt in range(LT):
                    ttg = b * LT + lt
                    ot = ostage.tile([128, D], HF)
                    for pairidx in range(D // 1024):
                        ps_o = ps_big.tile([128, 1024], FP32, tag="mm1024")
                        for half in range(2):
                            ec = pairidx * 2 + half
                            for hh in range(HL):
                                nc.tensor.matmul(
                                    ps_o[:, half * 512:(half + 1) * 512],
                                    yT_sb[:, hh, b, lt * 128:(lt + 1) * 128],
                                    wo_sb[:, hh, ec * 512:(ec + 1) * 512],
                                    start=(hh == 0), stop=(hh == HL - 1),
                                )
                        copy(
                            eng(c_flip),
                            ot[:, pairidx * 1024:(pairidx + 1) * 1024],
                            ps_o[:],
                        )
                        c_flip += 1
                    nc.sync.dma_start(
                        out[ttg * 128:(ttg + 1) * 128, :], ot[:]
                    )

            for b in range(B):
                phase_b(b)
                phase_c(b)
    return nc


def _rope_tables(L, dh, LT):
    inv_freq = 1.0 / (ROPE_THETA ** (np.arange(0, dh, 2, dtype=np.float32) / dh))
    ang = np.arange(L, dtype=np.float32)[:, None] * inv_freq[None, :]  # [L, dh/2]
    cos = np.repeat(np.cos(ang), 2, axis=-1)                          # [L, dh]
    sin = np.repeat(np.sin(ang), 2, axis=-1)
    sgn = np.where(np.arange(dh) % 2 == 0, -1.0, 1.0).astype(np.float32)
    sinn = sin * sgn[None, :]
    # [L, dh] -> [128, LT, dh] with partition = l % 128
    cosn = np.ascontiguousarray(
        cos.reshape(LT, 128, dh).transpose(1, 0, 2)
    ).astype(F16)
    sinn = np.ascontiguousarray(
        sinn.reshape(LT, 128, dh).transpose(1, 0, 2)
    ).astype(F16)
    return cosn, sinn


def _amask():
    # amask.T @ I accumulated onto a diagonal scores block adds
    # MASK_NEG where k > q: (amask.T @ I)[k, q] = amask[q_row...,]
    # out[i, j] += sum_p amask[p, i] * ident[p, j] = amask[j, i]
    # want out[k, q] += MASK_NEG for k > q  ->  amask[q, k] = MASK_NEG
    # for k > q (strict upper triangle).
    m = np.zeros((128, 128), dtype=F16)
    iu = np.triu_indices(128, k=1)
    m[iu] = MASK_NEG
    return m


def make_in_maps(x, W_qkv, W_o, n_cores=8, H=16):
    B, L, D = x.shape
    T = B * L
    dh = D // H
    HL = H // n_cores
    LT = L // 128
    xbfT = np.ascontiguousarray(x.reshape(T, D).T).astype(F16)
    cosn, sinn = _rope_tables(L, dh, LT)
    amask = _amask()
    identity = np.eye(128, dtype=F16)
    in_maps = []
    for c in range(n_cores):
        r0 = c * HL * dh
        r1 = (c + 1) * HL * dh
        wl = np.concatenate(
            [W_qkv[r0:r1], W_qkv[D + r0:D + r1], W_qkv[2 * D + r0:2 * D + r1]], axis=0
        )
        wqkvT = np.ascontiguousarray(wl.T).astype(F16)
        woT = np.ascontiguousarray(W_o[:, r0:r1].T).astype(F16)
        in_maps.append(
            {
                "xbT": xbfT,
                "wqkvT": wqkvT,
                "woT": woT,
                "cosn": cosn,
                "sinn": sinn,
                "amask": amask,
                "ident": identity,
            }
        )
    return in_maps


_NC_CACHE = {}


def _get_nc(B, L, D, HL):
    key = (B, L, D, HL)
    if key not in _NC_CACHE:
        _NC_CACHE[key] = build_core_kernel(B, L, D, HL)
    return _NC_CACHE[key]


def kernel(x, W_qkv, W_o, trace=False):
    x = np.asarray(x)
    W_qkv = np.asarray(W_qkv)
    W_o = np.asarray(W_o)
    B, L, D = x.shape
    n_cores, H = 8, 16
    HL = H // n_cores
    nc = _get_nc(B, L, D, HL)
    in_maps = make_in_maps(x, W_qkv, W_o, n_cores=n_cores, H=H)
    res = run_bass_kernel_spmd(
        nc, in_maps, core_ids=list(range(n_cores)), trace=trace
    )
    acc = np.zeros((B * L, D), dtype=np.float64)
    for r in res.results:
        acc += r["out"].astype(np.float64)
    out = acc.astype(np.float32).reshape(B, L, D)
    if trace:
        return out, res
    return out
